# revision 44
# baseline (speedup 1.0000x reference)
"""DistanceAwareAttention TRN2 kernel.

Module: y = LayerNorm(x + OutProj(MHA(x, bias=-|lam|*dist)))
Shapes: x [4, 2048, 1024], dist [2048, 2048], H=16 heads, D=64.

Sharding (8 cores, no collectives): core c handles (batch b = c//2,
query-half h = c%2).  Each core computes K/V for all 2048 keys of its
batch but Q/attention/output only for its 1024 query rows; output
blocks are disjoint, so the host just concatenates.

Device dataflow (per core, feature-on-partition "T" layouts):
  QT/KT [e_out, s] = W^T.T @ xT        (head pairs packed on partitions)
  V     [s, e_out] natural, +ones column per head (softmax denominator)
  scores^T [sk, sq] = KhT.T @ QhT      (per head, K=64, 2 heads row-packed)
  P^T = exp(scores/8) * Wd,  Wd = exp(-|lam|*dist)  (additive bias folded
        multiplicatively: ACT exp w/ scale=1/8, DVE/GpSimd multiply)
  O^T|denom = [Vh|1].T @ P^T           (denominator rides PSUM row 64)
  y[sq, e] = OT.T @ woT + (x + b_out)  (swapped-arg matmul -> natural
        layout, no transposes), LayerNorm along free dim, DMA out.
The attention / normalize / out-proj+LN stages are pipelined per
512-column sq block so PE never waits on the normalization bounce.
"""

import numpy as np
from contextlib import ExitStack

B, S, E, H, D = 4, 2048, 1024, 16, 64
P = 128
SQ = S // 2          # query rows per core
EP = E // P          # 8 e-tiles
SKT = S // P         # 16 sk tiles
NQ = SQ // 512       # 2 sq column blocks of 512
PAIRS = H // 2       # 8 head pairs
NCORES = 8


def _build_program(lam_abs: float, dbg: bool = False):
    import concourse.bass as bass
    import concourse.tile as tile
    from concourse import bacc, mybir
    from concourse.tile import add_dep_helper

    dt = mybir.dt
    f32, bf16 = dt.float32, dt.bfloat16
    AF = mybir.ActivationFunctionType
    ALU = mybir.AluOpType

    nc = bacc.Bacc(
        "TRN2",
        target_bir_lowering=False,
        debug=False,
        enable_asserts=False,
        num_devices=NCORES,
    )

    # ---- DRAM I/O ----
    xT_d = nc.dram_tensor("xT", [E, S], bf16, kind="ExternalInput").ap()
    xqb_d = nc.dram_tensor("xqb", [SQ, E], f32, kind="ExternalInput").ap()
    dist_d = nc.dram_tensor("distq", [S, SQ], bf16, kind="ExternalInput").ap()
    wqT_d = nc.dram_tensor("wqT", [E, E], bf16, kind="ExternalInput").ap()
    wkT_d = nc.dram_tensor("wkT", [E, E], bf16, kind="ExternalInput").ap()
    wvT_d = nc.dram_tensor("wvT", [E, E], bf16, kind="ExternalInput").ap()
    woT_d = nc.dram_tensor("woT", [E, E], bf16, kind="ExternalInput").ap()
    bq_d = nc.dram_tensor("bq", [E], f32, kind="ExternalInput").ap()
    bk_d = nc.dram_tensor("bk", [E], f32, kind="ExternalInput").ap()
    bv_d = nc.dram_tensor("bv", [E], bf16, kind="ExternalInput").ap()
    gamma_d = nc.dram_tensor("gamma", [E], f32, kind="ExternalInput").ap()
    beta_d = nc.dram_tensor("beta", [E], f32, kind="ExternalInput").ap()
    y_d = nc.dram_tensor("y", [SQ, E], f32, kind="ExternalOutput").ap()
    rd_dram = nc.dram_tensor("rd_scratch", [H, SQ], f32).ap()
    rd2_dram = nc.dram_tensor("rd2_scratch", [H, SQ], f32).ap()

    with tile.TileContext(nc) as tc, ExitStack() as ctx:
        const = ctx.enter_context(tc.tile_pool(name="const", bufs=1))
        persist = ctx.enter_context(tc.tile_pool(name="persist", bufs=1))

        # ---- constants ----
        ones1 = const.tile([1, P], bf16)
        nc.gpsimd.memset(ones1[:], 1.0)
        bq_sb = const.tile([P, EP], f32)
        nc.sync.dma_start(bq_sb[:], bq_d.rearrange("(o p) -> p o", p=P))
        bk_sb = const.tile([P, EP], f32)
        nc.sync.dma_start(bk_sb[:], bk_d.rearrange("(o p) -> p o", p=P))
        bv_sb = const.tile([1, E], bf16)
        nc.sync.dma_start(bv_sb[:], bv_d[None, :])
        eps_sb = const.tile([P, 1], f32)
        nc.gpsimd.memset(eps_sb[:], 1e-5)

        # ---- persistent tensors ----
        QT = persist.tile([P, PAIRS, SQ], bf16)
        KT = persist.tile([P, PAIRS, S], bf16)
        Vp = persist.tile([P, SKT, H, D + 1], bf16)
        OT = persist.tile([P, EP, SQ], bf16)
        Wd = persist.tile([P, SKT, SQ], bf16)

        nc.gpsimd.memset(Vp[:, :, :, D : D + 1], 1.0)

        # ================= Phase B: QKV projections =================
        xT_r = xT_d.rearrange("(o p) s -> p o s", p=P)
        wq_r = wqT_d.rearrange("(o p) e -> p o e", p=P)
        wk_r = wkT_d.rearrange("(o p) e -> p o e", p=P)
        wv_r = wvT_d.rearrange("(o p) e -> p o e", p=P)
        dist_r = dist_d.rearrange("(o p) q -> p o q", p=P)

        def qk_proj(w, dst, bias, ncols, ppB, xT):
            for j in range(EP):
                for n in range(ncols // 512):
                    ps = ppB.tile([P, 512], f32, tag="psB")
                    for k in range(EP):
                        nc.tensor.matmul(
                            ps[:],
                            w[:, k, j * P : (j + 1) * P],
                            xT[:, k, n * 512 : (n + 1) * 512],
                            start=(k == 0),
                            stop=(k == EP - 1),
                        )
                    nc.vector.tensor_scalar_add(
                        dst[:, j, n * 512 : (n + 1) * 512],
                        ps[:],
                        bias[:, j : j + 1],
                    )

        with tc.tile_pool(name="xtp", bufs=1) as xpool:
            xTk = [xpool.tile([P, S], bf16, tag=f"xT{k}", name=f"xT{k}") for k in range(EP)]
            for k in range(EP):
                nc.sync.dma_start(xTk[k][:], xT_r[:, k, :])

            class _KView:
                def __init__(self, tiles):
                    self.tiles = tiles

                def __getitem__(self, idx):
                    _, k, sl = idx
                    return self.tiles[k][:, sl]

            xT = _KView(xTk)

            with tc.tile_pool(name="wvq", bufs=1) as wpool, \
                 tc.tile_pool(name="ppB", bufs=4, space="PSUM") as ppB:
                wvk = [wpool.tile([P, E], bf16, tag=f"wv{k}", name=f"wv{k}") for k in range(EP)]
                wqk = [wpool.tile([P, E], bf16, tag=f"wq{k}", name=f"wq{k}") for k in range(EP)]
                wv, wq = _KView(wvk), _KView(wqk)
                for k in range(EP):
                    nc.sync.dma_start(wvk[k][:], wv_r[:, k, :])
                for k in range(EP):
                    nc.sync.dma_start(wqk[k][:], wq_r[:, k, :])
                # distance weights: DMA + exp early (ACT is idle here)
                for c in range(8):
                    sl = slice(c * 2, (c + 1) * 2)
                    nc.sync.dma_start(Wd[:, sl, :], dist_r[:, sl, :])
                    nc.scalar.activation(
                        Wd[:, sl, :], Wd[:, sl, :], AF.Exp, scale=-lam_abs
                    )

                # V natural: psum [sk 128, e_out 512]
                for t in range(SKT):
                    for n in range(2):
                        ps = ppB.tile([P, 512], f32, tag="psB")
                        for k in range(EP):
                            nc.tensor.matmul(
                                ps[:],
                                xT[:, k, t * P : (t + 1) * P],
                                wv[:, k, n * 512 : (n + 1) * 512],
                                start=(k == 0),
                                stop=False,
                            )
                        nc.tensor.matmul(
                            ps[:],
                            ones1[:1, :],
                            bv_sb[:1, n * 512 : (n + 1) * 512],
                            start=False,
                            stop=True,
                        )
                        nc.vector.tensor_copy(
                            Vp[:, t, 8 * n : 8 * (n + 1), 0:D],
                            ps.rearrange("p (h d) -> p h d", d=D),
                        )
                qk_proj(wq, QT, bq_sb, SQ, ppB, xT)

            with tc.tile_pool(name="wkp", bufs=1) as wpool2, \
                 tc.tile_pool(name="ppB2", bufs=4, space="PSUM") as ppB2:
                wkk = [wpool2.tile([P, E], bf16, tag=f"wk{k}", name=f"wk{k}") for k in range(EP)]
                wk = _KView(wkk)
                for k in range(EP):
                    nc.sync.dma_start(wkk[k][:], wk_r[:, k, :])
                qk_proj(wk, KT, bk_sb, S, ppB2, xT)

        # ====== Phase C: attention + normalize + out-proj + LN,
        #        pipelined per 512-wide sq block ======
        with tc.tile_pool(name="wd", bufs=1) as wdpool, \
             tc.tile_pool(name="wo", bufs=1) as wopool, \
             tc.tile_pool(name="norm", bufs=1) as normp, \
             tc.tile_pool(name="stC", bufs=2) as stC, \
             tc.tile_pool(name="stD", bufs=2) as stD, \
             tc.tile_pool(name="ynp", bufs=2) as ynp, \
             tc.tile_pool(name="stat", bufs=4) as stat, \
             tc.tile_pool(name="psS", bufs=2, space="PSUM") as psS, \
             tc.tile_pool(name="psO", bufs=1, space="PSUM") as psO, \
             tc.tile_pool(name="psD", bufs=2, space="PSUM") as psD:
            g_bc = wdpool.tile([P, E], f32, tag="gbc")
            nc.sync.dma_start(g_bc[:], gamma_d.partition_broadcast(P))
            be_bc = wdpool.tile([P, E], f32, tag="bebc")
            nc.sync.dma_start(be_bc[:], beta_d.partition_broadcast(P))
            wo = wopool.tile([P, EP, E], bf16)
            nc.sync.dma_start(wo[:], woT_d.rearrange("(o p) e -> p o e", p=P))
            xq_r = xqb_d.rearrange("(o p) e -> p o e", p=P)

            for nq in range(NQ):
                qs = nq * 512
                dn_dmas = []
                for pr in range(PAIRS):
                    hA, hB = 2 * pr, 2 * pr + 1
                    oAB = psO.tile([D + 1, 2, 512], f32, tag="oAB")
                    oA = oAB[:, 0, :]
                    oB = oAB[:, 1, :]
                    for t in range(SKT):
                        sAB = psS.tile([P, 2, 512], f32, tag="sAB")
                        nc.tensor.matmul(
                            sAB[:, 0, :],
                            KT[0:D, pr, t * P : (t + 1) * P],
                            QT[0:D, pr, qs : qs + 512],
                            start=True,
                            stop=True,
                        )
                        nc.tensor.matmul(
                            sAB[:, 1, :],
                            KT[D:P, pr, t * P : (t + 1) * P],
                            QT[D:P, pr, qs : qs + 512],
                            start=True,
                            stop=True,
                        )
                        eAB = stC.tile([P, 2, 512], bf16, tag="eAB")
                        nc.scalar.activation(eAB[:], sAB[:], AF.Exp, scale=0.125)
                        pAB = stC.tile([P, 2, 512], bf16, tag="pAB")
                        nc.vector.tensor_tensor(
                            pAB[:],
                            eAB[:],
                            Wd[:, t : t + 1, qs : qs + 512].to_broadcast(
                                (P, 2, 512)
                            ),
                            ALU.mult,
                        )
                        nc.tensor.matmul(
                            oA[:],
                            Vp[:, t, hA, :],
                            pAB[:, 0, :],
                            start=(t == 0),
                            stop=(t == SKT - 1),
                        )
                        nc.tensor.matmul(
                            oB[:],
                            Vp[:, t, hB, :],
                            pAB[:, 1, :],
                            start=(t == 0),
                            stop=(t == SKT - 1),
                        )
                    nc.vector.tensor_copy(OT[0:D, pr, qs : qs + 512], oA[0:D, :])
                    nc.vector.tensor_copy(OT[D:P, pr, qs : qs + 512], oB[0:D, :])
                    dn = stC.tile([1, 2, 512], f32, tag="dn")
                    nc.scalar.activation(dn[0:1, 0, :], oA[D : D + 1, :], AF.Copy)
                    nc.scalar.activation(dn[0:1, 1, :], oB[D : D + 1, :], AF.Copy)
                    dn_dmas.append(
                        nc.sync.dma_start(
                            rd_dram[hA : hB + 1, qs : qs + 512].rearrange(
                                "h q -> () h q"
                            ),
                            dn[0:1, :, :],
                        )
                    )

                # normalize this sq block: re-partition the denominators to
                # 128 lanes via DRAM (engines address partition bases
                # 0/32/64/96 only; recip on 1 lane is ~8cyc/elem), then
                # broadcast the reciprocals back through DRAM (0-stride
                # partition APs need a DRAM source).
                rec = normp.tile([P, 64], f32, tag="rec")
                rd_blk = rd_dram[:, qs : qs + 512].rearrange(
                    "h (a b) -> h a b", b=64
                )
                rd_read = nc.sync.dma_start(rec[:], rd_blk)
                for dma in dn_dmas:
                    add_dep_helper(rd_read.ins, dma.ins, reason="rd RAW")
                nc.vector.reciprocal(rec[:], rec[:])
                wr = nc.sync.dma_start(
                    rd2_dram[:, qs : qs + 512].rearrange("h (a b) -> h a b", b=64),
                    rec[:],
                )
                rdb = normp.tile([P, EP, 512], bf16, tag="rdb")
                for h in range(H):
                    j, g = h // 2, h % 2
                    bc = nc.gpsimd.dma_start(
                        rdb[g * D : (g + 1) * D, j, :],
                        rd2_dram[h : h + 1, qs : qs + 512].to_broadcast((D, 512)),
                    )
                    add_dep_helper(bc.ins, wr.ins, reason="rd2 RAW bc")
                nc.vector.tensor_tensor(
                    OT[:, :, qs : qs + 512], OT[:, :, qs : qs + 512], rdb[:],
                    ALU.mult,
                )

                # out-proj (swapped args -> natural [sq, e] layout) +
                # residual(+b_out, host-folded) + LayerNorm
                for m in range(4):
                    sqt = nq * 4 + m
                    yt = ynp.tile([P, E], f32, tag="yt")
                    xq = stD.tile([P, E], f32, tag="xq")
                    nc.sync.dma_start(xq[:], xq_r[:, sqt, :])
                    psm = psD.tile([P, 512], f32, tag="psD")
                    for n in range(2):
                        ps = psD.tile([P, 512], f32, tag="psD", name="psm2") if n else psm
                        for k in range(EP):
                            nc.tensor.matmul(
                                ps[:],
                                OT[:, k, sqt * P : (sqt + 1) * P],
                                wo[:, k, n * 512 : (n + 1) * 512],
                                start=(k == 0),
                                stop=(k == EP - 1),
                            )
                        nc.vector.tensor_tensor(
                            yt[:, n * 512 : (n + 1) * 512],
                            ps[:],
                            xq[:, n * 512 : (n + 1) * 512],
                            ALU.add,
                        )
                    stats = stat.tile([P, 2, 6], f32, tag="stats")
                    for sgi in range(2):
                        nc.vector.bn_stats(
                            stats[:, sgi, :], yt[:, sgi * 512 : (sgi + 1) * 512]
                        )
                    mv = stat.tile([P, 2], f32, tag="mv")
                    nc.vector.bn_aggr(mv[:], stats[:])
                    rstd = stat.tile([P, 1], f32, tag="rstd")
                    nc.scalar.activation(
                        rstd[:], mv[:, 1:2], AF.Sqrt, bias=eps_sb[:]
                    )
                    nc.vector.reciprocal(rstd[:], rstd[:])
                    nc.vector.tensor_scalar(
                        yt[:], yt[:], mv[:, 0:1], rstd[:], ALU.subtract, ALU.mult
                    )
                    nc.vector.tensor_tensor(yt[:], yt[:], g_bc[:], ALU.mult)
                    nc.vector.tensor_tensor(yt[:], yt[:], be_bc[:], ALU.add)
                    nc.sync.dma_start(
                        y_d.rearrange("(o p) e -> p o e", p=P)[:, sqt, :], yt[:]
                    )

    nc.compile()
    return nc


_prog_cache = {}
_last_in_maps = None


def _prep_in_maps(**inputs):
    import ml_dtypes

    x = np.asarray(inputs["x"], np.float32)
    dist = np.asarray(inputs["dist"], np.float32)
    w_qkv = np.asarray(inputs["w_qkv"], np.float32)
    b_qkv = np.asarray(inputs["b_qkv"], np.float32)
    w_out = np.asarray(inputs["w_out"], np.float32)
    b_out = np.asarray(inputs["b_out"], np.float32)
    gamma = np.asarray(inputs["gamma"], np.float32)
    beta = np.asarray(inputs["beta"], np.float32)

    bf = ml_dtypes.bfloat16
    wqT = np.ascontiguousarray(w_qkv[0:E].T).astype(bf)
    wkT = np.ascontiguousarray(w_qkv[E : 2 * E].T).astype(bf)
    wvT = np.ascontiguousarray(w_qkv[2 * E : 3 * E].T).astype(bf)
    woT = np.ascontiguousarray(w_out.T).astype(bf)
    bq, bk = b_qkv[0:E].copy(), b_qkv[E : 2 * E].copy()
    bv = b_qkv[2 * E : 3 * E].astype(bf)

    in_maps = []
    for c in range(NCORES):
        b, half = divmod(c, 2)
        qs = half * SQ
        perm = np.concatenate(
            [np.arange(qs, qs + SQ), np.arange(0, qs), np.arange(qs + SQ, S)]
        )
        xT = np.ascontiguousarray(x[b].T)  # [E, S]
        in_maps.append(
            {
                "xT": np.ascontiguousarray(xT[:, perm]).astype(bf),
                "xqb": np.ascontiguousarray(x[b, qs : qs + SQ, :]) + b_out,
                "distq": np.ascontiguousarray(
                    dist[perm][:, qs : qs + SQ]
                ).astype(bf),
                "wqT": wqT,
                "wkT": wkT,
                "wvT": wvT,
                "woT": woT,
                "bq": bq,
                "bk": bk,
                "bv": bv,
                "gamma": gamma,
                "beta": beta,
            }
        )
    return in_maps


def kernel(**inputs) -> np.ndarray:
    global _last_in_maps
    from concourse.bass_utils import run_bass_kernel_spmd
    from concourse.bass_interp import get_hw_module

    lam_abs = float(abs(np.float32(inputs["lam"])))
    in_maps = _prep_in_maps(**inputs)
    _last_in_maps = in_maps

    key = round(lam_abs, 9)
    if key not in _prog_cache:
        _prog_cache[key] = _build_program(lam_abs)
    nc = _prog_cache[key]

    old_m = nc.m
    nc.m = get_hw_module(nc.m)
    try:
        res = run_bass_kernel_spmd(nc, in_maps, list(range(NCORES)))
    finally:
        nc.m = old_m

    out = np.empty((B, S, E), np.float32)
    for c in range(NCORES):
        b, half = divmod(c, 2)
        out[b, half * SQ : (half + 1) * SQ, :] = res.results[c]["y"]
    return out


# revision 45
# speedup vs baseline: 1.0618x; 1.0618x over previous
"""DistanceAwareAttention TRN2 kernel.

Module: y = LayerNorm(x + OutProj(MHA(x, bias=-|lam|*dist)))
Shapes: x [4, 2048, 1024], dist [2048, 2048], H=16 heads, D=64.

Sharding (8 cores, no collectives): core c handles (batch b = c//2,
query-half h = c%2).  Each core computes K/V for all 2048 keys of its
batch but Q/attention/output only for its 1024 query rows; output
blocks are disjoint, so the host just concatenates.

Device dataflow (per core, feature-on-partition "T" layouts):
  QT/KT [e_out, s] = W^T.T @ xT        (head pairs packed on partitions)
  V     [s, e_out] natural, +ones column per head (softmax denominator)
  scores^T [sk, sq] = KhT.T @ QhT      (per head, K=64, 2 heads row-packed)
  P^T = exp(scores/8) * Wd,  Wd = exp(-|lam|*dist)  (additive bias folded
        multiplicatively: ACT exp w/ scale=1/8, DVE/GpSimd multiply)
  O^T|denom = [Vh|1].T @ P^T           (denominator rides PSUM row 64)
  y[sq, e] = OT.T @ woT + (x + b_out)  (swapped-arg matmul -> natural
        layout, no transposes), LayerNorm along free dim, DMA out.
The attention / normalize / out-proj+LN stages are pipelined per
512-column sq block so PE never waits on the normalization bounce.
"""

import numpy as np
from contextlib import ExitStack

B, S, E, H, D = 4, 2048, 1024, 16, 64
P = 128
SQ = S // 2          # query rows per core
EP = E // P          # 8 e-tiles
SKT = S // P         # 16 sk tiles
NQ = SQ // 512       # 2 sq column blocks of 512
PAIRS = H // 2       # 8 head pairs
NCORES = 8


def _build_program(lam_abs: float, use_v_bias: bool = True, dbg: bool = False):
    import concourse.bass as bass
    import concourse.tile as tile
    from concourse import bacc, mybir
    from concourse.tile import add_dep_helper

    dt = mybir.dt
    f32, bf16 = dt.float32, dt.bfloat16
    AF = mybir.ActivationFunctionType
    ALU = mybir.AluOpType

    nc = bacc.Bacc(
        "TRN2",
        target_bir_lowering=False,
        debug=False,
        enable_asserts=False,
        num_devices=NCORES,
    )

    # ---- DRAM I/O ----
    xT_d = nc.dram_tensor("xT", [E, S], bf16, kind="ExternalInput").ap()
    xqb_d = nc.dram_tensor("xqb", [SQ, E], f32, kind="ExternalInput").ap()
    dist_d = nc.dram_tensor("distq", [S, SQ], bf16, kind="ExternalInput").ap()
    wqT_d = nc.dram_tensor("wqT", [E, E], bf16, kind="ExternalInput").ap()
    wkT_d = nc.dram_tensor("wkT", [E, E], bf16, kind="ExternalInput").ap()
    wvT_d = nc.dram_tensor("wvT", [E, E], bf16, kind="ExternalInput").ap()
    woT_d = nc.dram_tensor("woT", [E, E], bf16, kind="ExternalInput").ap()
    bq_d = nc.dram_tensor("bq", [E], f32, kind="ExternalInput").ap()
    bk_d = nc.dram_tensor("bk", [E], f32, kind="ExternalInput").ap()
    bv_d = nc.dram_tensor("bv", [E], bf16, kind="ExternalInput").ap()
    gamma_d = nc.dram_tensor("gamma", [E], f32, kind="ExternalInput").ap()
    beta_d = nc.dram_tensor("beta", [E], f32, kind="ExternalInput").ap()
    y_d = nc.dram_tensor("y", [SQ, E], f32, kind="ExternalOutput").ap()
    rd_dram = nc.dram_tensor("rd_scratch", [H, SQ], f32).ap()
    rd2_dram = nc.dram_tensor("rd2_scratch", [H, SQ], f32).ap()

    with tile.TileContext(nc) as tc, ExitStack() as ctx:
        const = ctx.enter_context(tc.tile_pool(name="const", bufs=1))
        persist = ctx.enter_context(tc.tile_pool(name="persist", bufs=1))

        # ---- constants ----
        ones1 = const.tile([1, P], bf16)
        nc.gpsimd.memset(ones1[:], 1.0)
        bq_sb = const.tile([P, EP], f32)
        nc.sync.dma_start(bq_sb[:], bq_d.rearrange("(o p) -> p o", p=P))
        bk_sb = const.tile([P, EP], f32)
        nc.sync.dma_start(bk_sb[:], bk_d.rearrange("(o p) -> p o", p=P))
        bv_sb = const.tile([1, E], bf16)
        nc.sync.dma_start(bv_sb[:], bv_d[None, :])
        eps_sb = const.tile([P, 1], f32)
        nc.gpsimd.memset(eps_sb[:], 1e-5)

        # ---- persistent tensors ----
        QT = persist.tile([P, PAIRS, SQ], bf16)
        KT = persist.tile([P, PAIRS, S], bf16)
        Vp = persist.tile([P, SKT, H, D + 1], bf16)
        OT = persist.tile([P, EP, SQ], bf16)
        Wd = persist.tile([P, SKT, SQ], bf16)

        nc.gpsimd.memset(Vp[:, :, :, D : D + 1], 1.0)

        # ================= Phase B: QKV projections =================
        xT_r = xT_d.rearrange("(o p) s -> p o s", p=P)
        wq_r = wqT_d.rearrange("(o p) e -> p o e", p=P)
        wk_r = wkT_d.rearrange("(o p) e -> p o e", p=P)
        wv_r = wvT_d.rearrange("(o p) e -> p o e", p=P)
        dist_r = dist_d.rearrange("(o p) q -> p o q", p=P)

        def qk_proj(w, dst, bias, ncols, ppB, xT):
            for j in range(EP):
                for n in range(ncols // 512):
                    ps = ppB.tile([P, 512], f32, tag="psB")
                    for k in range(EP):
                        nc.tensor.matmul(
                            ps[:],
                            w[:, k, j * P : (j + 1) * P],
                            xT[:, k, n * 512 : (n + 1) * 512],
                            start=(k == 0),
                            stop=(k == EP - 1),
                        )
                    nc.vector.tensor_scalar_add(
                        dst[:, j, n * 512 : (n + 1) * 512],
                        ps[:],
                        bias[:, j : j + 1],
                    )

        with tc.tile_pool(name="xtp", bufs=1) as xpool:
            xTk = [xpool.tile([P, S], bf16, tag=f"xT{k}", name=f"xT{k}") for k in range(EP)]

            class _KView:
                def __init__(self, tiles):
                    self.tiles = tiles

                def __getitem__(self, idx):
                    _, k, sl = idx
                    return self.tiles[k][:, sl]

            xT = _KView(xTk)

            with tc.tile_pool(name="wvq", bufs=1) as wpool, \
                 tc.tile_pool(name="ppB", bufs=4, space="PSUM") as ppB:
                wvk = [wpool.tile([P, E], bf16, tag=f"wv{k}", name=f"wv{k}") for k in range(EP)]
                wqk = [wpool.tile([P, E], bf16, tag=f"wq{k}", name=f"wq{k}") for k in range(EP)]
                wv, wq = _KView(wvk), _KView(wqk)
                for k in range(EP):
                    nc.scalar.dma_start(wvk[k][:], wv_r[:, k, :])
                    nc.sync.dma_start(xTk[k][:], xT_r[:, k, :])
                for k in range(EP):
                    nc.scalar.dma_start(wqk[k][:], wq_r[:, k, :])
                # distance weights: DMA + exp early (ACT is idle here)
                for c in range(8):
                    sl = slice(c * 2, (c + 1) * 2)
                    nc.sync.dma_start(Wd[:, sl, :], dist_r[:, sl, :])
                    nc.scalar.activation(
                        Wd[:, sl, :], Wd[:, sl, :], AF.Exp, scale=-lam_abs
                    )

                # V natural: psum [sk 128, e_out 512]
                for t in range(SKT):
                    for n in range(2):
                        ps = ppB.tile([P, 512], f32, tag="psB")
                        for k in range(EP):
                            nc.tensor.matmul(
                                ps[:],
                                xT[:, k, t * P : (t + 1) * P],
                                wv[:, k, n * 512 : (n + 1) * 512],
                                start=(k == 0),
                                stop=(not use_v_bias and k == EP - 1),
                            )
                        if use_v_bias:
                            nc.tensor.matmul(
                                ps[:],
                                ones1[:1, :],
                                bv_sb[:1, n * 512 : (n + 1) * 512],
                                start=False,
                                stop=True,
                            )
                        nc.vector.tensor_copy(
                            Vp[:, t, 8 * n : 8 * (n + 1), 0:D],
                            ps.rearrange("p (h d) -> p h d", d=D),
                        )
                qk_proj(wq, QT, bq_sb, SQ, ppB, xT)

            with tc.tile_pool(name="wkp", bufs=1) as wpool2, \
                 tc.tile_pool(name="ppB2", bufs=4, space="PSUM") as ppB2:
                wkk = [wpool2.tile([P, E], bf16, tag=f"wk{k}", name=f"wk{k}") for k in range(EP)]
                wk = _KView(wkk)
                for k in range(EP):
                    nc.sync.dma_start(wkk[k][:], wk_r[:, k, :])
                qk_proj(wk, KT, bk_sb, S, ppB2, xT)

        # ====== Phase C: attention + normalize + out-proj + LN,
        #        pipelined per 512-wide sq block ======
        with tc.tile_pool(name="wd", bufs=1) as wdpool, \
             tc.tile_pool(name="wo", bufs=1) as wopool, \
             tc.tile_pool(name="norm", bufs=1) as normp, \
             tc.tile_pool(name="stC", bufs=2) as stC, \
             tc.tile_pool(name="stD", bufs=2) as stD, \
             tc.tile_pool(name="ynp", bufs=2) as ynp, \
             tc.tile_pool(name="stat", bufs=4) as stat, \
             tc.tile_pool(name="psS", bufs=2, space="PSUM") as psS, \
             tc.tile_pool(name="psO", bufs=1, space="PSUM") as psO, \
             tc.tile_pool(name="psD", bufs=2, space="PSUM") as psD:
            g_bc = wdpool.tile([P, E], f32, tag="gbc")
            nc.sync.dma_start(g_bc[:], gamma_d.partition_broadcast(P))
            be_bc = wdpool.tile([P, E], f32, tag="bebc")
            nc.sync.dma_start(be_bc[:], beta_d.partition_broadcast(P))
            wo = wopool.tile([P, EP, E], bf16)
            nc.sync.dma_start(wo[:], woT_d.rearrange("(o p) e -> p o e", p=P))
            xq_r = xqb_d.rearrange("(o p) e -> p o e", p=P)

            for nq in range(NQ):
                qs = nq * 512
                for pr in range(PAIRS):
                    hA, hB = 2 * pr, 2 * pr + 1
                    oAB = psO.tile([D + 1, 2, 512], f32, tag="oAB")
                    oA = oAB[:, 0, :]
                    oB = oAB[:, 1, :]
                    for t in range(SKT):
                        sAB = psS.tile([P, 2, 512], f32, tag="sAB")
                        nc.tensor.matmul(
                            sAB[:, 0, :],
                            KT[0:D, pr, t * P : (t + 1) * P],
                            QT[0:D, pr, qs : qs + 512],
                            start=True,
                            stop=True,
                        )
                        nc.tensor.matmul(
                            sAB[:, 1, :],
                            KT[D:P, pr, t * P : (t + 1) * P],
                            QT[D:P, pr, qs : qs + 512],
                            start=True,
                            stop=True,
                        )
                        eAB = stC.tile([P, 2, 512], bf16, tag="eAB")
                        nc.scalar.activation(eAB[:], sAB[:], AF.Exp, scale=0.125)
                        pAB = stC.tile([P, 2, 512], bf16, tag="pAB")
                        nc.vector.tensor_tensor(
                            pAB[:],
                            eAB[:],
                            Wd[:, t : t + 1, qs : qs + 512].to_broadcast(
                                (P, 2, 512)
                            ),
                            ALU.mult,
                        )
                        nc.tensor.matmul(
                            oA[:],
                            Vp[:, t, hA, :],
                            pAB[:, 0, :],
                            start=(t == 0),
                            stop=(t == SKT - 1),
                        )
                        nc.tensor.matmul(
                            oB[:],
                            Vp[:, t, hB, :],
                            pAB[:, 1, :],
                            start=(t == 0),
                            stop=(t == SKT - 1),
                        )
                    nc.vector.tensor_copy(OT[0:D, pr, qs : qs + 512], oA[0:D, :])
                    nc.vector.tensor_copy(OT[D:P, pr, qs : qs + 512], oB[0:D, :])
                    # per-pair normalization: denominators to DRAM, read
                    # back re-partitioned onto 128 lanes (engines address
                    # partition bases 0/32/64/96 only; recip is ~8cyc/elem
                    # per lane), reciprocal, bounce out, partition-broadcast
                    # back (0-stride APs need a DRAM source).
                    dn = stC.tile([1, 2, 512], f32, tag="dn")
                    nc.scalar.activation(dn[0:1, 0, :], oA[D : D + 1, :], AF.Copy)
                    nc.scalar.activation(dn[0:1, 1, :], oB[D : D + 1, :], AF.Copy)
                    dwr = nc.sync.dma_start(
                        rd_dram[hA : hB + 1, qs : qs + 512].rearrange(
                            "h q -> () h q"
                        ),
                        dn[0:1, :, :],
                    )
                    rec = normp.tile([P, 8], f32, tag="rec")
                    drd = nc.sync.dma_start(
                        rec[:],
                        rd_dram[hA : hB + 1, qs : qs + 512].rearrange(
                            "h (a b) -> h a b", b=8
                        ),
                    )
                    add_dep_helper(drd.ins, dwr.ins, reason="rd RAW")
                    nc.vector.reciprocal(rec[:], rec[:])
                    wr = nc.sync.dma_start(
                        rd2_dram[hA : hB + 1, qs : qs + 512].rearrange(
                            "h (a b) -> h a b", b=8
                        ),
                        rec[:],
                    )
                    rdb = normp.tile([P, 512], bf16, tag="rdb")
                    for g in range(2):
                        bc = nc.gpsimd.dma_start(
                            rdb[g * D : (g + 1) * D, :],
                            rd2_dram[
                                hA + g : hA + g + 1, qs : qs + 512
                            ].to_broadcast((D, 512)),
                        )
                        add_dep_helper(bc.ins, wr.ins, reason="rd2 RAW bc")
                    nc.vector.tensor_tensor(
                        OT[:, pr, qs : qs + 512],
                        OT[:, pr, qs : qs + 512],
                        rdb[:],
                        ALU.mult,
                    )

                # out-proj (swapped args -> natural [sq, e] layout) +
                # residual(+b_out, host-folded) + LayerNorm
                for m in range(4):
                    sqt = nq * 4 + m
                    yt = ynp.tile([P, E], f32, tag="yt")
                    xq = stD.tile([P, E], f32, tag="xq")
                    nc.sync.dma_start(xq[:], xq_r[:, sqt, :])
                    psm = psD.tile([P, 512], f32, tag="psD")
                    for n in range(2):
                        ps = psD.tile([P, 512], f32, tag="psD", name="psm2") if n else psm
                        for k in range(EP):
                            nc.tensor.matmul(
                                ps[:],
                                OT[:, k, sqt * P : (sqt + 1) * P],
                                wo[:, k, n * 512 : (n + 1) * 512],
                                start=(k == 0),
                                stop=(k == EP - 1),
                            )
                        nc.vector.tensor_tensor(
                            yt[:, n * 512 : (n + 1) * 512],
                            ps[:],
                            xq[:, n * 512 : (n + 1) * 512],
                            ALU.add,
                        )
                    stats = stat.tile([P, 2, 6], f32, tag="stats")
                    for sgi in range(2):
                        nc.vector.bn_stats(
                            stats[:, sgi, :], yt[:, sgi * 512 : (sgi + 1) * 512]
                        )
                    mv = stat.tile([P, 2], f32, tag="mv")
                    nc.vector.bn_aggr(mv[:], stats[:])
                    rstd = stat.tile([P, 1], f32, tag="rstd")
                    nc.scalar.activation(
                        rstd[:], mv[:, 1:2], AF.Sqrt, bias=eps_sb[:]
                    )
                    nc.vector.reciprocal(rstd[:], rstd[:])
                    nc.vector.tensor_scalar(
                        yt[:], yt[:], mv[:, 0:1], rstd[:], ALU.subtract, ALU.mult
                    )
                    nc.vector.tensor_tensor(yt[:], yt[:], g_bc[:], ALU.mult)
                    nc.vector.tensor_tensor(yt[:], yt[:], be_bc[:], ALU.add)
                    nc.sync.dma_start(
                        y_d.rearrange("(o p) e -> p o e", p=P)[:, sqt, :], yt[:]
                    )

    nc.compile()
    return nc


_prog_cache = {}
_last_in_maps = None


def _prep_in_maps(**inputs):
    import ml_dtypes

    x = np.asarray(inputs["x"], np.float32)
    dist = np.asarray(inputs["dist"], np.float32)
    w_qkv = np.asarray(inputs["w_qkv"], np.float32)
    b_qkv = np.asarray(inputs["b_qkv"], np.float32)
    w_out = np.asarray(inputs["w_out"], np.float32)
    b_out = np.asarray(inputs["b_out"], np.float32)
    gamma = np.asarray(inputs["gamma"], np.float32)
    beta = np.asarray(inputs["beta"], np.float32)

    bf = ml_dtypes.bfloat16
    wqT = np.ascontiguousarray(w_qkv[0:E].T).astype(bf)
    wkT = np.ascontiguousarray(w_qkv[E : 2 * E].T).astype(bf)
    wvT = np.ascontiguousarray(w_qkv[2 * E : 3 * E].T).astype(bf)
    woT = np.ascontiguousarray(w_out.T).astype(bf)
    bq, bk = b_qkv[0:E].copy(), b_qkv[E : 2 * E].copy()
    bv = b_qkv[2 * E : 3 * E].astype(bf)

    in_maps = []
    for c in range(NCORES):
        b, half = divmod(c, 2)
        qs = half * SQ
        perm = np.concatenate(
            [np.arange(qs, qs + SQ), np.arange(0, qs), np.arange(qs + SQ, S)]
        )
        xT = np.ascontiguousarray(x[b].T)  # [E, S]
        in_maps.append(
            {
                "xT": np.ascontiguousarray(xT[:, perm]).astype(bf),
                "xqb": np.ascontiguousarray(x[b, qs : qs + SQ, :]) + b_out,
                "distq": np.ascontiguousarray(
                    dist[perm][:, qs : qs + SQ]
                ).astype(bf),
                "wqT": wqT,
                "wkT": wkT,
                "wvT": wvT,
                "woT": woT,
                "bq": bq,
                "bk": bk,
                "bv": bv,
                "gamma": gamma,
                "beta": beta,
            }
        )
    return in_maps


def kernel(**inputs) -> np.ndarray:
    global _last_in_maps
    from concourse.bass_utils import run_bass_kernel_spmd
    from concourse.bass_interp import get_hw_module

    lam_abs = float(abs(np.float32(inputs["lam"])))
    in_maps = _prep_in_maps(**inputs)
    _last_in_maps = in_maps

    use_v_bias = bool(np.any(np.asarray(inputs["b_qkv"], np.float32)[2 * E : 3 * E]))
    key = (round(lam_abs, 9), use_v_bias)
    if key not in _prog_cache:
        _prog_cache[key] = _build_program(lam_abs, use_v_bias)
    nc = _prog_cache[key]

    old_m = nc.m
    nc.m = get_hw_module(nc.m)
    try:
        res = run_bass_kernel_spmd(nc, in_maps, list(range(NCORES)))
    finally:
        nc.m = old_m

    out = np.empty((B, S, E), np.float32)
    for c in range(NCORES):
        b, half = divmod(c, 2)
        out[b, half * SQ : (half + 1) * SQ, :] = res.results[c]["y"]
    return out
